# revision 17
# baseline (speedup 1.0000x reference)
"""Trainium2 Bass kernel for nn_CardGNN (3-layer GATv2 message passing), v3.

Sharding: nodes partitioned across 8 NeuronCores (6250 nodes each, padded to
6272 = 49 blocks x 128 dst nodes). The xr source-transform is computed by the
OWNING core only and AllGathered node-major, so the collective output buffer
IS the dma_gather table (no per-core table recompute / scatter-store).
Self-loop edges bypass the gather via an SBUF-resident copy of the core's own
xr rows. Gather index streams are padded with -1 (trimmed by the SWDGE ucode).
One-hot matrices for x_i expansion and segment-sum accumulation are
precomputed host-side per (core, block) and streamed from DRAM.
"""
import math
import os
import numpy as np
import ml_dtypes

KDBG_NOGATHER = os.environ.get("KDBG_NOGATHER", "0") == "1"
KDBG_NOCOLL = os.environ.get("KDBG_NOCOLL", "0") == "1"
KDBG_LAYERS = int(os.environ.get("KDBG_LAYERS", "3"))

import concourse.bacc as bacc
import concourse.mybir as mybir
import concourse.tile as tile
from concourse.bass_utils import run_bass_kernel_spmd

F32 = mybir.dt.float32
BF16 = mybir.dt.bfloat16
F8 = mybir.dt.float8e4
I16 = mybir.dt.int16
AF = mybir.ActivationFunctionType
OP = mybir.AluOpType

N = 50000
IN = 128
HID = 32
HEADS = 4
CH = 32
HC = HID * HEADS  # 128
EMB = 128
NCORES = 8
NPC = N // NCORES          # 6250 real nodes per core
P = 128
BN = 128                   # dst nodes per block
NBLK = 49                  # ceil(6250/128)
NPAD = NBLK * BN           # 6272
ROWS = NCORES * NPAD       # 50176 table rows (node-major, padded per core)
SPLIT = 32768              # lo/hi table split so gather idx fits int16
NEG = 0.2
LAYERS = 3


def _row_of(g):
    return (g // NPC) * NPAD + (g % NPC)


def _wrap16(idx_flat):
    """dma_gather index layout: w[p, s] = idx[s*16+p%16], replicated to 128 rows."""
    w = idx_flat.reshape(-1, 16).T.astype(np.int16)
    return np.tile(w, (8, 1))


def _preprocess(edge_index):
    src = np.asarray(edge_index[0]).astype(np.int64)
    dst = np.asarray(edge_index[1]).astype(np.int64)
    order = np.argsort(dst, kind="stable")
    src = src[order]
    dst = dst[order]

    core = dst // NPC
    dloc = dst - core * NPC
    blk = dloc // BN
    dl = dloc - blk * BN           # dst offset within block, 0..127
    gb = core * NBLK + blk         # global block id 0..391
    srow = _row_of(src)

    NGB = NCORES * NBLK
    starts = np.searchsorted(gb, np.arange(NGB))
    ends = np.searchsorted(gb, np.arange(NGB) + 1)

    lo_lists, hi_lists = [], []
    maxlo = maxhi = 0
    for g in range(NGB):
        s, e = starts[g], ends[g]
        sr, dg = srow[s:e], dl[s:e]
        m = sr < SPLIT
        lo_lists.append((sr[m], dg[m]))
        hi_lists.append((sr[~m] - SPLIT, dg[~m]))
        maxlo = max(maxlo, int(m.sum()))
        maxhi = max(maxhi, int((~m).sum()))
    NL = max(1, math.ceil(maxlo / P))
    NH = max(1, math.ceil(maxhi / P))
    SUBS = 1 + NL + NH             # self sub + lo subs + hi subs

    idx_lo = np.full((NCORES, NBLK, P, NL * 8), -1, np.int16)
    idx_hi = np.full((NCORES, NBLK, P, NH * 8), -1, np.int16)
    # one-hots: st[p, j, q] = (dl[j*128+p] == q)  (acc stationary)
    #           s_t[p, s]   = (dl[s] == p)        (x_i expansion stationary)
    st_oh = np.zeros((NCORES, NBLK, P, SUBS, P), ml_dtypes.float8_e4m3fn)
    s_oh = np.zeros((NCORES, NBLK, P, SUBS * P), ml_dtypes.float8_e4m3fn)

    eye = np.eye(P, dtype=np.float32)
    for g in range(NGB):
        c, b = divmod(g, NBLK)
        (lsr, ldl), (hsr, hdl) = lo_lists[g], hi_lists[g]
        jl = np.full(NL * P, 0, np.int64); jl[:len(lsr)] = lsr
        jh = np.full(NH * P, 0, np.int64); jh[:len(hsr)] = hsr
        idx_lo[c, b] = _wrap16(jl)
        idx_hi[c, b] = _wrap16(jh)
        # dl vector over all SUBS*128 slots; -1 = invalid
        dlv = np.full(SUBS * P, -1, np.int64)
        nself = min(NPC - b * BN, BN)           # valid self-loop dst count
        dlv[:nself] = np.arange(nself)
        dlv[P:P + len(ldl)] = ldl
        dlv[(1 + NL) * P:(1 + NL) * P + len(hdl)] = hdl
        valid = dlv >= 0
        oh = np.zeros((SUBS * P, P), np.float32)
        oh[valid] = eye[dlv[valid]]
        st_oh[c, b] = oh.reshape(SUBS, P, P).transpose(1, 0, 2).astype(ml_dtypes.float8_e4m3fn)
        s_oh[c, b] = oh.T.astype(ml_dtypes.float8_e4m3fn)
    return dict(NL=NL, NH=NH, SUBS=SUBS, idx_lo=idx_lo, idx_hi=idx_hi,
                st_oh=st_oh, s_oh=s_oh)


def _bcast(v, rows=P):
    v = np.asarray(v, np.float32).reshape(-1)
    return np.tile(v[None, :], (rows, 1)).astype(np.float32)


def _bcast16(v, rows=P):
    return _bcast(v, rows).astype(ml_dtypes.bfloat16)


def _build(NL, NH):
    SUBS = 1 + NL + NH
    NG = NL + NH                  # gathered subs
    NCH = 5                       # x_i psum chunks per block
    SUBC = (SUBS + NCH - 1) // NCH
    nc = bacc.Bacc()

    # ---- I/O ----
    xT_ext = nc.declare_dram_parameter("xT", [IN, NPAD], F32, isOutput=False)
    G8 = (NL + NH) * 8
    ixall_ext = nc.declare_dram_parameter("ixall", [NBLK, P, G8], I16, isOutput=False)
    stoh_ext = nc.declare_dram_parameter("stoh", [NBLK, P, SUBS * P], F8, isOutput=False)
    soh_ext = nc.declare_dram_parameter("soh", [NBLK, P, SUBS * P], F8, isOutput=False)
    win_ext = nc.declare_dram_parameter("win", [IN, HID], F32, isOutput=False)
    binb_ext = nc.declare_dram_parameter("binb", [P, HID], F32, isOutput=False)
    wl_ext, wr_ext, blb_ext, brb_ext, attb_ext, bob_ext, gb_ext, beb_ext = [], [], [], [], [], [], [], []
    for i in range(LAYERS):
        ic = HID if i == 0 else HC
        wl_ext.append(nc.declare_dram_parameter(f"wl{i}", [ic, HC], F32, isOutput=False))
        wr_ext.append(nc.declare_dram_parameter(f"wr{i}", [ic, HC], F32, isOutput=False))
        blb_ext.append(nc.declare_dram_parameter(f"blb{i}", [P, HC], F32, isOutput=False))
        brb_ext.append(nc.declare_dram_parameter(f"brb{i}", [P, HC], F32, isOutput=False))
        attb_ext.append(nc.declare_dram_parameter(f"attb{i}", [P, HC], BF16, isOutput=False))
        bob_ext.append(nc.declare_dram_parameter(f"bob{i}", [P, HC], F32, isOutput=False))
        gb_ext.append(nc.declare_dram_parameter(f"gb{i}", [P, HC], F32, isOutput=False))
        beb_ext.append(nc.declare_dram_parameter(f"beb{i}", [P, HC], F32, isOutput=False))
    wout_ext = nc.declare_dram_parameter("wout", [HC, EMB], F32, isOutput=False)
    boutb_ext = nc.declare_dram_parameter("boutb", [P, EMB], F32, isOutput=False)
    out_ext = nc.declare_dram_parameter("out", [NPC, EMB], F32, isOutput=True)

    with tile.TileContext(nc) as tc:
        with (
            tc.tile_pool(name="dram", bufs=1, space="DRAM") as dpool,
            tc.tile_pool(name="pers", bufs=1) as pers,
            tc.tile_pool(name="wpool", bufs=1) as wpool,
            tc.tile_pool(name="work", bufs=1) as work,
            tc.tile_pool(name="ework", bufs=2) as ework,
            tc.tile_pool(name="gbuf", bufs=2) as gbuf,
            tc.tile_pool(name="small", bufs=2) as small,
            tc.tile_pool(name="psA", bufs=2, space="PSUM") as psA,
            tc.tile_pool(name="psX", bufs=2, space="PSUM") as psX,
            tc.tile_pool(name="psB", bufs=2, space="PSUM") as psB,
        ):
            # ---- DRAM internals: allgather in/out per layer ----
            ag_in = [dpool.tile([NPAD, HC], BF16, tag=f"ag_in{j}", name=f"ag_in{j}")
                     for j in range(LAYERS)]
            ag_out = [dpool.tile([ROWS, HC], BF16, tag=f"ag_out{j}", name=f"ag_out{j}",
                                 addr_space="Shared") for j in range(LAYERS)]
            tab = [dpool.tile([ROWS, HC], BF16, tag=f"tab{j}", name=f"tab{j}")
                   for j in range(LAYERS)]

            # ---- persistent SBUF ----
            hT_a = pers.tile([P, NPAD], F32, tag="hT_a")      # node features, channel-major
            hT_b = pers.tile([P, NPAD], F32, tag="hT_b")
            xl_all = pers.tile([P, NBLK, HC], BF16, tag="xl_all")
            xr_self = pers.tile([P, NBLK, HC], BF16, tag="xr_self")
            id_t = pers.tile([P, P], F32, tag="ident")
            eps5_t = pers.tile([P, 1], F32, tag="eps5")
            acc_all = pers.tile([P, NBLK, HC + HEADS], F32, tag="acc_all")
            ix_all = pers.tile([P, NBLK, (NL + NH) * 8], I16, tag="ix_all")

            from concourse.masks import make_identity
            make_identity(nc, id_t[:])
            nc.vector.memset(eps5_t[:], 1e-5)

            # prime the rotating gather-dest buffers so trimmed (padded) slots
            # hold finite values on first use
            for _ in range(3):
                t = gbuf.tile([P, NG, HC], BF16, tag="xj", bufs=3)
                nc.vector.memset(t[:], 0.0)

            # ================= h0 = gelu(x @ W_in + b_in) =================
            nc.sync.dma_start(ix_all[:], ixall_ext[:].rearrange("b p k -> p b k"))
            xT_t = hT_b
            nc.sync.dma_start(xT_t[:], xT_ext[:])
            win_t = wpool.tile([IN, HID], F32, tag="win")
            binb_t = wpool.tile([P, HID], F32, tag="binb")
            nc.sync.dma_start(win_t[:], win_ext[:])
            nc.sync.dma_start(binb_t[:], binb_ext[:])
            for b in range(NBLK):
                cs = slice(b * BN, (b + 1) * BN)
                ps = psA.tile([P, HC], F32, tag="mm")
                nc.tensor.matmul(ps[:, :HID], xT_t[:IN, cs], win_t[:], start=True, stop=True)
                h0s = work.tile([P, HID], F32, tag="h0s")
                nc.vector.tensor_tensor(out=h0s[:], in0=ps[:, :HID], in1=binb_t[:], op=OP.add)
                h0g = work.tile([P, HID], F32, tag="h0g")
                nc.scalar.activation(h0g[:], h0s[:], AF.Gelu)
                tp = psA.tile([HC, P], F32, tag="tp")
                nc.tensor.transpose(tp[:HID, :], h0g[:], id_t[:])
                nc.vector.tensor_copy(hT_a[:HID, cs], tp[:HID, :])

            hT_prev, hT_new = hT_a, hT_b

            # per-layer weights loaded up front
            wl_t, wr_t, blb_t, brb_t, attb_t, bob_t, gb_t, beb_t = [], [], [], [], [], [], [], []
            for li in range(LAYERS):
                ic = HID if li == 0 else HC
                wl_t.append(wpool.tile([HC, HC], F32, tag=f"wl{li}", name=f"wl{li}"))
                wr_t.append(wpool.tile([HC, HC], F32, tag=f"wr{li}", name=f"wr{li}"))
                blb_t.append(wpool.tile([P, HC], F32, tag=f"blb{li}", name=f"blb{li}"))
                brb_t.append(wpool.tile([P, HC], F32, tag=f"brb{li}", name=f"brb{li}"))
                attb_t.append(wpool.tile([P, HC], BF16, tag=f"attb{li}", name=f"attb{li}"))
                bob_t.append(wpool.tile([P, HC], F32, tag=f"bob{li}", name=f"bob{li}"))
                gb_t.append(wpool.tile([P, HC], F32, tag=f"gb{li}", name=f"gb{li}"))
                beb_t.append(wpool.tile([P, HC], F32, tag=f"beb{li}", name=f"beb{li}"))
                nc.sync.dma_start(wl_t[li][:ic, :], wl_ext[li][:])
                nc.sync.dma_start(wr_t[li][:ic, :], wr_ext[li][:])
                nc.sync.dma_start(blb_t[li][:], blb_ext[li][:])
                nc.sync.dma_start(brb_t[li][:], brb_ext[li][:])
                nc.sync.dma_start(attb_t[li][:], attb_ext[li][:])
                nc.sync.dma_start(bob_t[li][:], bob_ext[li][:])
                nc.sync.dma_start(gb_t[li][:], gb_ext[li][:])
                nc.sync.dma_start(beb_t[li][:], beb_ext[li][:])

            def _xr_transform(li, hT_src):
                """xr = hT_src^T @ Wr + br for own nodes -> xr_self, then allgather."""
                ic = HID if li == 0 else HC
                for b in range(NBLK):
                    cs = slice(b * BN, (b + 1) * BN)
                    ps = psA.tile([P, HC], F32, tag="mm")
                    nc.tensor.matmul(ps[:], hT_src[:ic, cs], wr_t[li][:ic, :], start=True, stop=True)
                    nc.vector.tensor_tensor(out=xr_self[:, b, :], in0=ps[:], in1=brb_t[li][:], op=OP.add)
                nc.sync.dma_start(
                    ag_in[li][:].rearrange("(b p) c -> p b c", p=P), xr_self[:])
                if not KDBG_NOCOLL:
                    nc.gpsimd.collective_compute(
                        "AllGather", OP.bypass, replica_groups=[list(range(NCORES))],
                        ins=[ag_in[li].opt()], outs=[ag_out[li].opt()],
                    )
                    CK = ROWS // 8
                    engs = [nc.sync, nc.scalar]
                    for k in range(8):
                        engs[k % 2].dma_start(tab[li][k * CK:(k + 1) * CK, :],
                                              ag_out[li][k * CK:(k + 1) * CK, :])
                else:
                    for cc in range(NCORES):
                        nc.sync.dma_start(
                            ag_out[li][cc * NPAD:(cc + 1) * NPAD, :].rearrange(
                                "(b p) c -> p b c", p=P), xr_self[:])

            _xr_transform(0, hT_a)

            for li in range(KDBG_LAYERS):
                ic = HID if li == 0 else HC
                agout = ag_out[li]
                tabli = tab[li] if not KDBG_NOCOLL else ag_out[li]

                # ---- xl (own nodes) -> SBUF xl_all, bf16 ----
                for b in range(NBLK):
                    cs = slice(b * BN, (b + 1) * BN)
                    ps = psA.tile([P, HC], F32, tag="mm")
                    nc.tensor.matmul(ps[:], hT_prev[:ic, cs], wl_t[li][:ic, :], start=True, stop=True)
                    nc.vector.tensor_tensor(out=xl_all[:, b, :], in0=ps[:], in1=blb_t[li][:], op=OP.add)

                # ---- post-processing (softmax-normalize + LN + gelu + residual) ----
                def _post(b0, b1, li=li, hT_prev=hT_prev, hT_new=hT_new):
                    HB = b1 - b0
                    t_ap = acc_all[:, b0:b1, :HC]
                    den_t = small.tile([P, NBLK, HEADS], F32, tag="den", name="den_t")
                    nc.vector.tensor_scalar(out=den_t[:, :HB, :], in0=acc_all[:, b0:b1, HC:],
                                            scalar1=1e-16, scalar2=None, op0=OP.add)
                    rec_t = small.tile([P, NBLK, HEADS], F32, tag="rec", name="rec_t")
                    nc.vector.reciprocal(rec_t[:, :HB, :], den_t[:, :HB, :])
                    nc.vector.tensor_tensor(
                        out=t_ap.rearrange("p b (h c) -> p b h c", h=HEADS),
                        in0=t_ap.rearrange("p b (h c) -> p b h c", h=HEADS),
                        in1=rec_t[:, :HB, :, None].broadcast_to([P, HB, HEADS, CH]), op=OP.mult)
                    nc.vector.tensor_tensor(
                        out=t_ap, in0=t_ap,
                        in1=bob_t[li][:, None, :].broadcast_to([P, HB, HC]), op=OP.add)
                    mu_t = small.tile([P, NBLK], F32, tag="mu", name="mu_t")
                    nc.vector.reduce_sum(mu_t[:, :HB], t_ap, axis=mybir.AxisListType.X)
                    nc.vector.tensor_scalar(out=mu_t[:, :HB], in0=mu_t[:, :HB],
                                            scalar1=1.0 / HC, scalar2=None, op0=OP.mult)
                    nc.vector.tensor_tensor(
                        out=t_ap, in0=t_ap,
                        in1=mu_t[:, :HB, None].broadcast_to([P, HB, HC]), op=OP.subtract)
                    var_t = small.tile([P, NBLK], F32, tag="var", name="var_t")
                    sqs_t = small.tile([P, HC], F32, tag="sqs", name="sqs_t")
                    for b in range(b0, b1):
                        nc.scalar.activation(sqs_t[:], acc_all[:, b, :HC], AF.Square,
                                             accum_out=var_t[:, b - b0:b - b0 + 1])
                    std_t = small.tile([P, NBLK], F32, tag="std", name="std_t")
                    nc.scalar.activation(std_t[:, :HB], var_t[:, :HB], AF.Sqrt,
                                         scale=1.0 / HC, bias=eps5_t[:, :1])
                    rstd_t = small.tile([P, NBLK], F32, tag="rstd", name="rstd_t")
                    nc.vector.reciprocal(rstd_t[:, :HB], std_t[:, :HB])
                    nc.vector.tensor_tensor(
                        out=t_ap, in0=t_ap,
                        in1=rstd_t[:, :HB, None].broadcast_to([P, HB, HC]), op=OP.mult)
                    nc.vector.tensor_tensor(
                        out=t_ap, in0=t_ap,
                        in1=gb_t[li][:, None, :].broadcast_to([P, HB, HC]), op=OP.mult)
                    nc.vector.tensor_tensor(
                        out=t_ap, in0=t_ap,
                        in1=beb_t[li][:, None, :].broadcast_to([P, HB, HC]), op=OP.add)
                    nc.scalar.activation(t_ap, t_ap, AF.Gelu)
                    for b in range(b0, b1):
                        cs = slice(b * BN, (b + 1) * BN)
                        tp = psA.tile([HC, P], F32, tag="tp")
                        nc.tensor.transpose(tp[:, :], acc_all[:, b, :HC], id_t[:])
                        if li == 0:
                            nc.vector.tensor_copy(hT_new[:, cs], tp[:])
                        else:
                            nc.vector.tensor_tensor(out=hT_new[:, cs], in0=tp[:],
                                                    in1=hT_prev[:, cs], op=OP.add)

                # ---- edge blocks ----
                for b in range(NBLK):
                    if b == NBLK // 2 + 2:
                        _post(0, NBLK // 2)
                    st_t = gbuf.tile([P, SUBS, P], F8, tag="st")
                    s_t = gbuf.tile([P, SUBS * P], F8, tag="s")
                    nc.sync.dma_start(st_t[:], stoh_ext[b].rearrange("p (j q) -> p j q", q=P))
                    nc.sync.dma_start(s_t[:], soh_ext[b])

                    xj_t = gbuf.tile([P, NG, HC], BF16, tag="xj", bufs=3)
                    if KDBG_NOGATHER:
                        nc.vector.memset(xj_t[:], 0.0)
                    else:
                        nc.gpsimd.dma_gather(
                            out_ap=xj_t[:, :NL, :], in_ap=tabli[:SPLIT, :], idxs_ap=ix_all[:, b, :NL * 8],
                            num_idxs=NL * P, num_idxs_reg=NL * P, elem_size=HC,
                            single_packet=False)
                        nc.gpsimd.dma_gather(
                            out_ap=xj_t[:, NL:, :], in_ap=tabli[SPLIT:, :], idxs_ap=ix_all[:, b, NL * 8:],
                            num_idxs=NH * P, num_idxs_reg=NH * P, elem_size=HC,
                            single_packet=False)

                    # x_i expansion on PE; et = prelu(x_i + x_j)
                    et_t = ework.tile([P, SUBS, HC], BF16, tag="et")
                    for jc in range(NCH):
                        j0 = jc * SUBC
                        j1 = min(SUBS, j0 + SUBC)
                        if j0 >= j1:
                            continue
                        xi_ps = psX.tile([P, SUBC * HC], F32, tag="xi")
                        for j in range(j0, j1):
                            nc.tensor.matmul(
                                xi_ps[:, (j - j0) * HC:(j - j0 + 1) * HC],
                                s_t[:, j * P:(j + 1) * P], xl_all[:, b, :],
                                start=True, stop=True)
                        if j0 == 0:
                            nc.vector.tensor_tensor(
                                out=et_t[:, 0, :], in0=xi_ps[:, :HC],
                                in1=xr_self[:, b, :], op=OP.add)
                            nc.vector.tensor_tensor(
                                out=et_t[:, 1:j1, :],
                                in0=xi_ps[:, HC:(j1 - j0) * HC].rearrange("p (j c) -> p j c", c=HC),
                                in1=xj_t[:, 0:j1 - 1, :], op=OP.add)
                        else:
                            nc.vector.tensor_tensor(
                                out=et_t[:, j0:j1, :],
                                in0=xi_ps[:, :(j1 - j0) * HC].rearrange("p (j c) -> p j c", c=HC),
                                in1=xj_t[:, j0 - 1:j1 - 1, :], op=OP.add)
                    nc.scalar.activation(et_t[:], et_t[:], AF.Prelu, alpha=NEG)
                    nc.vector.tensor_tensor(
                        out=et_t[:], in0=et_t[:],
                        in1=attb_t[li][:, None, :].broadcast_to([P, SUBS, HC]), op=OP.mult)
                    lg_t = small.tile([P, SUBS, HEADS], F32, tag="lg")
                    nc.vector.reduce_sum(
                        lg_t[:], et_t[:].rearrange("p j (h c) -> p j h c", h=HEADS),
                        axis=mybir.AxisListType.X)
                    ex_t = small.tile([P, SUBS, HEADS], BF16, tag="ex")
                    nc.scalar.activation(ex_t[:], lg_t[:], AF.Exp)

                    v_t = ework.tile([P, SUBS, HC + HEADS], BF16, tag="v")
                    nc.vector.tensor_tensor(
                        out=v_t[:, 0, :HC].rearrange("p (h c) -> p h c", h=HEADS),
                        in0=xr_self[:, b, :].rearrange("p (h c) -> p h c", h=HEADS),
                        in1=ex_t[:, 0, :, None].broadcast_to([P, HEADS, CH]), op=OP.mult)
                    nc.vector.tensor_tensor(
                        out=v_t[:, 1:, :HC].rearrange("p j (h c) -> p j h c", h=HEADS),
                        in0=xj_t[:].rearrange("p j (h c) -> p j h c", h=HEADS),
                        in1=ex_t[:, 1:, :, None].broadcast_to([P, NG, HEADS, CH]), op=OP.mult)
                    nc.vector.tensor_copy(v_t[:, :, HC:], ex_t[:])

                    acc = psB.tile([P, HC + HEADS], F32, tag="acc")
                    for j in range(SUBS):
                        nc.tensor.matmul(acc[:], st_t[:, j, :], v_t[:, j, :],
                                         start=(j == 0), stop=(j == SUBS - 1))
                    nc.vector.tensor_copy(acc_all[:, b, :], acc[:])

                _post(NBLK // 2, NBLK)

                hT_prev, hT_new = hT_new, hT_prev
                if li < KDBG_LAYERS - 1:
                    _xr_transform(li + 1, hT_prev)

            # ================= out = normalize(h @ W_out + b_out) =================
            wout_t = wpool.tile([HC, EMB], F32, tag="wout")
            boutb_t = wpool.tile([P, EMB], F32, tag="boutb")
            nc.sync.dma_start(wout_t[:], wout_ext[:])
            nc.sync.dma_start(boutb_t[:], boutb_ext[:])
            for b in range(NBLK):
                rows = min(NPC - b * BN, BN)
                cs = slice(b * BN, (b + 1) * BN)
                ps = psA.tile([P, EMB], F32, tag="mm")
                nc.tensor.matmul(ps[:], hT_prev[:HC, cs], wout_t[:], start=True, stop=True)
                osb = work.tile([P, EMB], F32, tag="osb")
                nc.vector.tensor_tensor(out=osb[:], in0=ps[:], in1=boutb_t[:], op=OP.add)
                sq_t = work.tile([P, EMB], F32, tag="osq")
                nsq_t = small.tile([P, 1], F32, tag="nsq")
                nc.scalar.activation(sq_t[:], osb[:], AF.Square, accum_out=nsq_t[:, :1])
                nrm_t = small.tile([P, 1], F32, tag="nrm")
                nc.scalar.activation(nrm_t[:], nsq_t[:], AF.Sqrt)
                nc.vector.tensor_scalar(out=nrm_t[:], in0=nrm_t[:], scalar1=1e-12,
                                        scalar2=None, op0=OP.max)
                recn_t = small.tile([P, 1], F32, tag="recn")
                nc.vector.reciprocal(recn_t[:], nrm_t[:])
                nc.vector.tensor_scalar(out=osb[:], in0=osb[:], scalar1=recn_t[:, :1],
                                        scalar2=None, op0=OP.mult)
                nc.sync.dma_start(out_ext[b * BN:b * BN + rows, :], osb[:rows, :])

    nc.compile()
    return nc


def _make_in_maps(inputs, meta):
    x = np.asarray(inputs["x"], np.float32)
    common = {
        "win": np.asarray(inputs["W_in"], np.float32),
        "binb": _bcast(inputs["b_in"]),
        "wout": np.asarray(inputs["W_out"], np.float32),
        "boutb": _bcast(inputs["b_out"]),
    }
    for i in range(LAYERS):
        common[f"wl{i}"] = np.asarray(inputs[f"Wl{i}"], np.float32)
        common[f"wr{i}"] = np.asarray(inputs[f"Wr{i}"], np.float32)
        common[f"blb{i}"] = _bcast(inputs[f"bl{i}"])
        common[f"brb{i}"] = _bcast(inputs[f"br{i}"])
        common[f"attb{i}"] = _bcast16(np.asarray(inputs[f"att{i}"], np.float32).reshape(-1))
        common[f"bob{i}"] = _bcast(inputs[f"bo{i}"])
        common[f"gb{i}"] = _bcast(inputs[f"g{i}"])
        common[f"beb{i}"] = _bcast(inputs[f"be{i}"])
    SUBS = meta["SUBS"]
    in_maps = []
    for c in range(NCORES):
        m = dict(common)
        xT = np.zeros((IN, NPAD), np.float32)
        xT[:, :NPC] = x[c * NPC:(c + 1) * NPC, :].T
        m["xT"] = xT
        m["ixall"] = np.concatenate([meta["idx_lo"][c], meta["idx_hi"][c]], axis=-1)
        m["stoh"] = meta["st_oh"][c].reshape(NBLK, P, SUBS * P)
        m["soh"] = meta["s_oh"][c]
        in_maps.append(m)
    return in_maps


def kernel(**inputs):
    edge_index = np.asarray(inputs["edge_index"])
    meta = _preprocess(edge_index)
    nc = _build(meta["NL"], meta["NH"])
    in_maps = _make_in_maps(inputs, meta)
    res = run_bass_kernel_spmd(nc, in_maps, list(range(NCORES)))
    out = np.concatenate([res.results[c]["out"] for c in range(NCORES)], axis=0)
    return out.astype(np.float32)


# revision 27
# speedup vs baseline: 1.5275x; 1.5275x over previous
"""Trainium2 Bass kernel for nn_CardGNN (3-layer GATv2 message passing), v3.

Sharding: nodes partitioned across 8 NeuronCores (6250 nodes each, padded to
6272 = 49 blocks x 128 dst nodes). The xr source-transform is computed by the
OWNING core only and AllGathered node-major, so the collective output buffer
IS the dma_gather table (no per-core table recompute / scatter-store).
Self-loop edges bypass the gather via an SBUF-resident copy of the core's own
xr rows. Gather index streams are padded with -1 (trimmed by the SWDGE ucode).
One-hot matrices for x_i expansion and segment-sum accumulation are
precomputed host-side per (core, block) and streamed from DRAM.
"""
import math
import os
import numpy as np
import ml_dtypes

KDBG_NOGATHER = os.environ.get("KDBG_NOGATHER", "0") == "1"
KDBG_NOCOLL = os.environ.get("KDBG_NOCOLL", "0") == "1"
KDBG_LAYERS = int(os.environ.get("KDBG_LAYERS", "3"))

import concourse.bacc as bacc
import concourse.mybir as mybir
import concourse.tile as tile
from concourse.bass_utils import run_bass_kernel_spmd

F32 = mybir.dt.float32
BF16 = mybir.dt.bfloat16
F8 = mybir.dt.float8e4
I16 = mybir.dt.int16
AF = mybir.ActivationFunctionType
OP = mybir.AluOpType

N = 50000
IN = 128
HID = 32
HEADS = 4
CH = 32
HC = HID * HEADS  # 128
EMB = 128
NCORES = 8
NPC = N // NCORES          # 6250 real nodes per core
P = 128
BN = 128                   # dst nodes per block
NBLK = 49                  # ceil(6250/128)
NPAD = NBLK * BN           # 6272
ROWS = NCORES * NPAD       # 50176 table rows (node-major, padded per core)
SPLIT = 32768              # lo/hi table split so gather idx fits int16
NEG = 0.2
LAYERS = 3


def _row_of(g):
    return (g // NPC) * NPAD + (g % NPC)


def _wrap16(idx_flat):
    """dma_gather index layout: w[p, s] = idx[s*16+p%16], replicated to 128 rows."""
    w = idx_flat.reshape(-1, 16).T.astype(np.int16)
    return np.tile(w, (8, 1))


def _preprocess(edge_index):
    src = np.asarray(edge_index[0]).astype(np.int64)
    dst = np.asarray(edge_index[1]).astype(np.int64)
    order = np.argsort(dst, kind="stable")
    src = src[order]
    dst = dst[order]

    core = dst // NPC
    dloc = dst - core * NPC
    blk = dloc // BN
    dl = dloc - blk * BN           # dst offset within block, 0..127
    gb = core * NBLK + blk         # global block id 0..391
    srow = _row_of(src)

    NGB = NCORES * NBLK
    starts = np.searchsorted(gb, np.arange(NGB))
    ends = np.searchsorted(gb, np.arange(NGB) + 1)

    lo_lists, hi_lists = [], []
    cnt_lo = np.zeros((NCORES, NBLK), np.int64)
    cnt_hi = np.zeros((NCORES, NBLK), np.int64)
    for g in range(NGB):
        s, e = starts[g], ends[g]
        sr, dg = srow[s:e], dl[s:e]
        m = sr < SPLIT
        lo_lists.append((sr[m], dg[m]))
        hi_lists.append((sr[~m] - SPLIT, dg[~m]))
        c, b = divmod(g, NBLK)
        cnt_lo[c, b] = int(m.sum())
        cnt_hi[c, b] = int((~m).sum())
    # per-block static sub counts (max over cores -> same program all cores)
    nlb = np.maximum(1, -(-cnt_lo.max(axis=0) // P)).astype(np.int64)   # [NBLK]
    nhb = np.maximum(1, -(-cnt_hi.max(axis=0) // P)).astype(np.int64)
    subs_b = 1 + nlb + nhb
    iofs = np.zeros(NBLK + 1, np.int64)   # idx array col offsets (int16 units)
    sofs = np.zeros(NBLK + 1, np.int64)   # one-hot col offsets
    for b in range(NBLK):
        iofs[b + 1] = iofs[b] + (nlb[b] + nhb[b]) * 8
        sofs[b + 1] = sofs[b] + subs_b[b] * P
    TI, TS = int(iofs[-1]), int(sofs[-1])

    ixall = np.zeros((NCORES, P, TI), np.int16)
    st_oh = np.zeros((NCORES, P, TS), ml_dtypes.float8_e4m3fn)
    s_oh = np.zeros((NCORES, P, TS), ml_dtypes.float8_e4m3fn)

    eye = np.eye(P, dtype=np.float32)
    for g in range(NGB):
        c, b = divmod(g, NBLK)
        NLb, NHb = int(nlb[b]), int(nhb[b])
        SUBSb = 1 + NLb + NHb
        (lsr, ldl), (hsr, hdl) = lo_lists[g], hi_lists[g]
        jl = np.full(NLb * P, 0, np.int64); jl[:len(lsr)] = lsr
        jh = np.full(NHb * P, 0, np.int64); jh[:len(hsr)] = hsr
        ixall[c, :, iofs[b]:iofs[b] + NLb * 8] = _wrap16(jl)
        ixall[c, :, iofs[b] + NLb * 8:iofs[b + 1]] = _wrap16(jh)
        dlv = np.full(SUBSb * P, -1, np.int64)
        nself = min(NPC - b * BN, BN)
        dlv[:nself] = np.arange(nself)
        dlv[P:P + len(ldl)] = ldl
        dlv[(1 + NLb) * P:(1 + NLb) * P + len(hdl)] = hdl
        valid = dlv >= 0
        oh = np.zeros((SUBSb * P, P), np.float32)
        oh[valid] = eye[dlv[valid]]
        st_oh[c, :, sofs[b]:sofs[b + 1]] = (
            oh.reshape(SUBSb, P, P).transpose(1, 0, 2).reshape(P, SUBSb * P)
            .astype(ml_dtypes.float8_e4m3fn))
        s_oh[c, :, sofs[b]:sofs[b + 1]] = oh.T.astype(ml_dtypes.float8_e4m3fn)
    return dict(nlb=nlb.tolist(), nhb=nhb.tolist(), iofs=iofs.tolist(),
                sofs=sofs.tolist(), TI=TI, TS=TS,
                ixall=ixall, st_oh=st_oh, s_oh=s_oh)


def _bcast(v, rows=P):
    v = np.asarray(v, np.float32).reshape(-1)
    return np.tile(v[None, :], (rows, 1)).astype(np.float32)


def _bcast16(v, rows=P):
    return _bcast(v, rows).astype(ml_dtypes.bfloat16)


def _build(meta):
    nlb, nhb = meta["nlb"], meta["nhb"]
    iofs, sofs = meta["iofs"], meta["sofs"]
    TI, TS = meta["TI"], meta["TS"]
    MAXS = max(1 + nlb[b] + nhb[b] for b in range(NBLK))
    MAXG = max(nlb[b] + nhb[b] for b in range(NBLK))
    SUBC = 4                      # subs per x_i psum chunk
    nc = bacc.Bacc()

    # ---- I/O ----
    xT_ext = nc.declare_dram_parameter("xT", [IN, NPAD], F32, isOutput=False)
    ixall_ext = nc.declare_dram_parameter("ixall", [P, TI], I16, isOutput=False)
    stoh_ext = nc.declare_dram_parameter("stoh", [P, TS], F8, isOutput=False)
    soh_ext = nc.declare_dram_parameter("soh", [P, TS], F8, isOutput=False)
    win_ext = nc.declare_dram_parameter("win", [IN, HID], F32, isOutput=False)
    binb_ext = nc.declare_dram_parameter("binb", [P, HID], F32, isOutput=False)
    wl_ext, wr_ext, blb_ext, brb_ext, attb_ext, bob_ext, gb_ext, beb_ext = [], [], [], [], [], [], [], []
    for i in range(LAYERS):
        ic = HID if i == 0 else HC
        wl_ext.append(nc.declare_dram_parameter(f"wl{i}", [ic, HC], F32, isOutput=False))
        wr_ext.append(nc.declare_dram_parameter(f"wr{i}", [ic, HC], F32, isOutput=False))
        blb_ext.append(nc.declare_dram_parameter(f"blb{i}", [P, HC], F32, isOutput=False))
        brb_ext.append(nc.declare_dram_parameter(f"brb{i}", [P, HC], F32, isOutput=False))
        attb_ext.append(nc.declare_dram_parameter(f"attb{i}", [P, HC], BF16, isOutput=False))
        bob_ext.append(nc.declare_dram_parameter(f"bob{i}", [P, HC], F32, isOutput=False))
        gb_ext.append(nc.declare_dram_parameter(f"gb{i}", [P, HC], F32, isOutput=False))
        beb_ext.append(nc.declare_dram_parameter(f"beb{i}", [P, HC], F32, isOutput=False))
    wout_ext = nc.declare_dram_parameter("wout", [HC, EMB], F32, isOutput=False)
    boutb_ext = nc.declare_dram_parameter("boutb", [P, EMB], F32, isOutput=False)
    out_ext = nc.declare_dram_parameter("out", [NPC, EMB], F32, isOutput=True)

    with tile.TileContext(nc) as tc:
        with (
            tc.tile_pool(name="dram", bufs=1, space="DRAM") as dpool,
            tc.tile_pool(name="pers", bufs=1) as pers,
            tc.tile_pool(name="wpool", bufs=1) as wpool,
            tc.tile_pool(name="work", bufs=1) as work,
            tc.tile_pool(name="ework", bufs=2) as ework,
            tc.tile_pool(name="gbuf", bufs=2) as gbuf,
            tc.tile_pool(name="small", bufs=2) as small,
            tc.tile_pool(name="psA", bufs=2, space="PSUM") as psA,
            tc.tile_pool(name="psX", bufs=2, space="PSUM") as psX,
            tc.tile_pool(name="psB", bufs=2, space="PSUM") as psB,
        ):
            # ---- DRAM internals: allgather in/out per layer ----
            ag_in = [dpool.tile([NPAD, HC], BF16, tag=f"ag_in{j}", name=f"ag_in{j}")
                     for j in range(LAYERS)]
            ag_out = [dpool.tile([ROWS, HC], BF16, tag=f"ag_out{j}", name=f"ag_out{j}",
                                 addr_space="Shared") for j in range(LAYERS)]

            # ---- persistent SBUF ----
            hT_a = pers.tile([P, NPAD], F32, tag="hT_a")      # node features, channel-major
            hT_b = pers.tile([P, NPAD], F32, tag="hT_b")
            xl_all = pers.tile([P, NBLK, HC], BF16, tag="xl_all")
            xr_self = pers.tile([P, NBLK, HC], BF16, tag="xr_self")
            id_t = pers.tile([P, P], F32, tag="ident")
            eps5_t = pers.tile([P, 1], F32, tag="eps5")
            acc_all = pers.tile([P, NBLK, HC + HEADS], F32, tag="acc_all")
            ix_all = pers.tile([P, TI], I16, tag="ix_all")

            from concourse.masks import make_identity
            make_identity(nc, id_t[:])
            nc.vector.memset(eps5_t[:], 1e-5)

            # prime the rotating gather-dest buffers so trimmed (padded) slots
            # hold finite values on first use
            for _ in range(3):
                t = gbuf.tile([P, MAXG, HC], BF16, tag="xj", bufs=3)
                nc.vector.memset(t[:], 0.0)

            # ================= h0 = gelu(x @ W_in + b_in) =================
            nc.sync.dma_start(ix_all[:], ixall_ext[:])
            xT_t = hT_b
            nc.sync.dma_start(xT_t[:], xT_ext[:])
            win_t = wpool.tile([IN, HID], F32, tag="win")
            binb_t = wpool.tile([P, HID], F32, tag="binb")
            nc.sync.dma_start(win_t[:], win_ext[:])
            nc.sync.dma_start(binb_t[:], binb_ext[:])
            for b in range(NBLK):
                cs = slice(b * BN, (b + 1) * BN)
                ps = psA.tile([P, HC], F32, tag="mm")
                nc.tensor.matmul(ps[:, :HID], xT_t[:IN, cs], win_t[:], start=True, stop=True)
                h0s = work.tile([P, HID], F32, tag="h0s")
                nc.vector.tensor_tensor(out=h0s[:], in0=ps[:, :HID], in1=binb_t[:], op=OP.add)
                h0g = work.tile([P, HID], F32, tag="h0g")
                nc.scalar.activation(h0g[:], h0s[:], AF.Gelu)
                tp = psA.tile([HC, P], F32, tag="tp")
                nc.tensor.transpose(tp[:HID, :], h0g[:], id_t[:])
                nc.vector.tensor_copy(hT_a[:HID, cs], tp[:HID, :])

            hT_prev, hT_new = hT_a, hT_b
            wout_t = wpool.tile([HC, EMB], F32, tag="wout")
            boutb_t = wpool.tile([P, EMB], F32, tag="boutb")
            nc.sync.dma_start(wout_t[:], wout_ext[:])
            nc.sync.dma_start(boutb_t[:], boutb_ext[:])

            # per-layer weights loaded up front
            wl_t, wr_t, blb_t, brb_t, attb_t, bob_t, gb_t, beb_t = [], [], [], [], [], [], [], []
            for li in range(LAYERS):
                ic = HID if li == 0 else HC
                wl_t.append(wpool.tile([HC, HC], F32, tag=f"wl{li}", name=f"wl{li}"))
                wr_t.append(wpool.tile([HC, HC], F32, tag=f"wr{li}", name=f"wr{li}"))
                blb_t.append(wpool.tile([P, HC], F32, tag=f"blb{li}", name=f"blb{li}"))
                brb_t.append(wpool.tile([P, HC], F32, tag=f"brb{li}", name=f"brb{li}"))
                attb_t.append(wpool.tile([P, HC], BF16, tag=f"attb{li}", name=f"attb{li}"))
                bob_t.append(wpool.tile([P, HC], F32, tag=f"bob{li}", name=f"bob{li}"))
                gb_t.append(wpool.tile([P, HC], F32, tag=f"gb{li}", name=f"gb{li}"))
                beb_t.append(wpool.tile([P, HC], F32, tag=f"beb{li}", name=f"beb{li}"))
                nc.sync.dma_start(wl_t[li][:ic, :], wl_ext[li][:])
                nc.sync.dma_start(wr_t[li][:ic, :], wr_ext[li][:])
                nc.sync.dma_start(blb_t[li][:], blb_ext[li][:])
                nc.sync.dma_start(brb_t[li][:], brb_ext[li][:])
                nc.sync.dma_start(attb_t[li][:], attb_ext[li][:])
                nc.sync.dma_start(bob_t[li][:], bob_ext[li][:])
                nc.sync.dma_start(gb_t[li][:], gb_ext[li][:])
                nc.sync.dma_start(beb_t[li][:], beb_ext[li][:])

            def _xr_transform(li, hT_src, b0=0, b1=NBLK, finish=True):
                """xr = hT_src^T @ Wr + br for own nodes -> xr_self, then allgather."""
                ic = HID if li == 0 else HC
                for b in range(b0, b1):
                    cs = slice(b * BN, (b + 1) * BN)
                    ps = psA.tile([P, HC], F32, tag="mm")
                    nc.tensor.matmul(ps[:], hT_src[:ic, cs], wr_t[li][:ic, :], start=True, stop=True)
                    nc.vector.tensor_tensor(out=xr_self[:, b, :], in0=ps[:], in1=brb_t[li][:], op=OP.add)
                if not finish:
                    return
                nc.sync.dma_start(
                    ag_in[li][:].rearrange("(b p) c -> p b c", p=P), xr_self[:])
                nc.gpsimd.collective_compute(
                    "AllGather", OP.bypass, replica_groups=[list(range(NCORES))],
                    ins=[ag_in[li].opt()], outs=[ag_out[li].opt()],
                )

            _xr_transform(0, hT_a)

            for li in range(KDBG_LAYERS):
                ic = HID if li == 0 else HC
                agout = ag_out[li]
                tabli = ag_out[li]

                # ---- xl (own nodes) -> SBUF xl_all, bf16 ----
                for b in range(NBLK):
                    cs = slice(b * BN, (b + 1) * BN)
                    ps = psA.tile([P, HC], F32, tag="mm")
                    nc.tensor.matmul(ps[:], hT_prev[:ic, cs], wl_t[li][:ic, :], start=True, stop=True)
                    nc.vector.tensor_tensor(out=xl_all[:, b, :], in0=ps[:], in1=blb_t[li][:], op=OP.add)

                # ---- post-processing (softmax-normalize + LN + gelu + residual) ----
                def _post(b0, b1, li=li, hT_prev=hT_prev, hT_new=hT_new):
                    HB = b1 - b0
                    t_ap = acc_all[:, b0:b1, :HC]
                    den_t = small.tile([P, NBLK, HEADS], F32, tag="den", name="den_t")
                    nc.vector.tensor_scalar(out=den_t[:, :HB, :], in0=acc_all[:, b0:b1, HC:],
                                            scalar1=1e-16, scalar2=None, op0=OP.add)
                    rec_t = small.tile([P, NBLK, HEADS], F32, tag="rec", name="rec_t")
                    nc.vector.reciprocal(rec_t[:, :HB, :], den_t[:, :HB, :])
                    nc.vector.tensor_tensor(
                        out=t_ap.rearrange("p b (h c) -> p b h c", h=HEADS),
                        in0=t_ap.rearrange("p b (h c) -> p b h c", h=HEADS),
                        in1=rec_t[:, :HB, :, None].broadcast_to([P, HB, HEADS, CH]), op=OP.mult)
                    nc.vector.tensor_tensor(
                        out=t_ap, in0=t_ap,
                        in1=bob_t[li][:, None, :].broadcast_to([P, HB, HC]), op=OP.add)
                    mu_t = small.tile([P, NBLK], F32, tag="mu", name="mu_t")
                    nc.vector.reduce_sum(mu_t[:, :HB], t_ap, axis=mybir.AxisListType.X)
                    nc.vector.tensor_scalar(out=mu_t[:, :HB], in0=mu_t[:, :HB],
                                            scalar1=1.0 / HC, scalar2=None, op0=OP.mult)
                    nc.vector.tensor_tensor(
                        out=t_ap, in0=t_ap,
                        in1=mu_t[:, :HB, None].broadcast_to([P, HB, HC]), op=OP.subtract)
                    var_t = small.tile([P, NBLK], F32, tag="var", name="var_t")
                    sqs_t = small.tile([P, HC], F32, tag="sqs", name="sqs_t")
                    for b in range(b0, b1):
                        nc.scalar.activation(sqs_t[:], acc_all[:, b, :HC], AF.Square,
                                             accum_out=var_t[:, b - b0:b - b0 + 1])
                    std_t = small.tile([P, NBLK], F32, tag="std", name="std_t")
                    nc.scalar.activation(std_t[:, :HB], var_t[:, :HB], AF.Sqrt,
                                         scale=1.0 / HC, bias=eps5_t[:, :1])
                    rstd_t = small.tile([P, NBLK], F32, tag="rstd", name="rstd_t")
                    nc.vector.reciprocal(rstd_t[:, :HB], std_t[:, :HB])
                    nc.vector.tensor_tensor(
                        out=t_ap, in0=t_ap,
                        in1=rstd_t[:, :HB, None].broadcast_to([P, HB, HC]), op=OP.mult)
                    nc.vector.tensor_tensor(
                        out=t_ap, in0=t_ap,
                        in1=gb_t[li][:, None, :].broadcast_to([P, HB, HC]), op=OP.mult)
                    nc.vector.tensor_tensor(
                        out=t_ap, in0=t_ap,
                        in1=beb_t[li][:, None, :].broadcast_to([P, HB, HC]), op=OP.add)
                    nc.scalar.activation(t_ap, t_ap, AF.Gelu)
                    for b in range(b0, b1):
                        cs = slice(b * BN, (b + 1) * BN)
                        tp = psA.tile([HC, P], F32, tag="tp")
                        nc.tensor.transpose(tp[:, :], acc_all[:, b, :HC], id_t[:])
                        if li == 0:
                            nc.vector.tensor_copy(hT_new[:, cs], tp[:])
                        else:
                            nc.vector.tensor_tensor(out=hT_new[:, cs], in0=tp[:],
                                                    in1=hT_prev[:, cs], op=OP.add)
                        if li == LAYERS - 1:
                            rows = min(NPC - b * BN, BN)
                            pso = psA.tile([P, EMB], F32, tag="mm", name="pso")
                            nc.tensor.matmul(pso[:], hT_new[:HC, cs], wout_t[:], start=True, stop=True)
                            osb = work.tile([P, EMB], F32, tag="osb")
                            nc.vector.tensor_tensor(out=osb[:], in0=pso[:], in1=boutb_t[:], op=OP.add)
                            sq_t = work.tile([P, EMB], F32, tag="osq")
                            nsq_t = small.tile([P, 1], F32, tag="nsq")
                            nc.scalar.activation(sq_t[:], osb[:], AF.Square, accum_out=nsq_t[:, :1])
                            nrm_t = small.tile([P, 1], F32, tag="nrm")
                            nc.scalar.activation(nrm_t[:], nsq_t[:], AF.Sqrt)
                            nc.vector.tensor_scalar(out=nrm_t[:], in0=nrm_t[:], scalar1=1e-12,
                                                    scalar2=None, op0=OP.max)
                            recn_t = small.tile([P, 1], F32, tag="recn")
                            nc.vector.reciprocal(recn_t[:], nrm_t[:])
                            nc.vector.tensor_scalar(out=osb[:], in0=osb[:], scalar1=recn_t[:, :1],
                                                    scalar2=None, op0=OP.mult)
                            nc.sync.dma_start(out_ext[b * BN:b * BN + rows, :], osb[:rows, :])

                # ---- edge blocks ----
                for b in range(NBLK):
                    NLb, NHb = nlb[b], nhb[b]
                    NGb = NLb + NHb
                    SUBSb = 1 + NGb
                    NCHb = (SUBSb + SUBC - 1) // SUBC
                    QT = NBLK // 4
                    if b in (QT + 2, 2 * QT + 2, 3 * QT + 2):
                        q0 = (b - QT - 2) // QT * QT
                        _post(q0, q0 + QT)
                        if li < KDBG_LAYERS - 1:
                            _xr_transform(li + 1, hT_new, q0, q0 + QT, finish=False)
                    st_t = gbuf.tile([P, MAXS, P], F8, tag="st")
                    s_t = gbuf.tile([P, MAXS * P], F8, tag="s")
                    nc.sync.dma_start(
                        st_t[:, :SUBSb, :],
                        stoh_ext[:, sofs[b]:sofs[b + 1]].rearrange("p (j q) -> p j q", q=P))
                    nc.sync.dma_start(s_t[:, :SUBSb * P], soh_ext[:, sofs[b]:sofs[b + 1]])

                    xj_t = gbuf.tile([P, MAXG, HC], BF16, tag="xj", bufs=3)
                    if KDBG_NOGATHER:
                        nc.vector.memset(xj_t[:], 0.0)
                    else:
                        nc.gpsimd.dma_gather(
                            out_ap=xj_t[:, :NLb, :], in_ap=agout[:SPLIT, :],
                            idxs_ap=ix_all[:, iofs[b]:iofs[b] + NLb * 8],
                            num_idxs=NLb * P, num_idxs_reg=NLb * P, elem_size=HC,
                            single_packet=False)
                        nc.gpsimd.dma_gather(
                            out_ap=xj_t[:, NLb:NGb, :], in_ap=agout[SPLIT:, :],
                            idxs_ap=ix_all[:, iofs[b] + NLb * 8:iofs[b + 1]],
                            num_idxs=NHb * P, num_idxs_reg=NHb * P, elem_size=HC,
                            single_packet=False)

                    # x_i expansion on PE; et = prelu(x_i + x_j)
                    et_t = ework.tile([P, MAXS, HC], BF16, tag="et")
                    for jc in range(NCHb):
                        j0 = jc * SUBC
                        j1 = min(SUBSb, j0 + SUBC)
                        if j0 >= j1:
                            continue
                        xi_ps = psX.tile([P, SUBC * HC], F32, tag="xi")
                        for j in range(j0, j1):
                            nc.tensor.matmul(
                                xi_ps[:, (j - j0) * HC:(j - j0 + 1) * HC],
                                s_t[:, j * P:(j + 1) * P], xl_all[:, b, :],
                                start=True, stop=True)
                        if j0 == 0:
                            nc.vector.tensor_tensor(
                                out=et_t[:, 0, :], in0=xi_ps[:, :HC],
                                in1=xr_self[:, b, :], op=OP.add)
                            nc.vector.tensor_tensor(
                                out=et_t[:, 1:j1, :],
                                in0=xi_ps[:, HC:(j1 - j0) * HC].rearrange("p (j c) -> p j c", c=HC),
                                in1=xj_t[:, 0:j1 - 1, :], op=OP.add)
                        else:
                            nc.vector.tensor_tensor(
                                out=et_t[:, j0:j1, :],
                                in0=xi_ps[:, :(j1 - j0) * HC].rearrange("p (j c) -> p j c", c=HC),
                                in1=xj_t[:, j0 - 1:j1 - 1, :], op=OP.add)
                    nc.scalar.activation(et_t[:, :SUBSb, :], et_t[:, :SUBSb, :], AF.Prelu, alpha=NEG)
                    nc.vector.tensor_tensor(
                        out=et_t[:, :SUBSb, :], in0=et_t[:, :SUBSb, :],
                        in1=attb_t[li][:, None, :].broadcast_to([P, SUBSb, HC]), op=OP.mult)
                    lg_t = small.tile([P, MAXS, HEADS], F32, tag="lg")
                    nc.vector.reduce_sum(
                        lg_t[:, :SUBSb, :], et_t[:, :SUBSb, :].rearrange("p j (h c) -> p j h c", h=HEADS),
                        axis=mybir.AxisListType.X)
                    ex_t = small.tile([P, MAXS, HEADS], BF16, tag="ex")
                    nc.scalar.activation(ex_t[:, :SUBSb, :], lg_t[:, :SUBSb, :], AF.Exp)

                    v_t = ework.tile([P, MAXS, HC + HEADS], BF16, tag="v")
                    nc.vector.tensor_tensor(
                        out=v_t[:, 0, :HC].rearrange("p (h c) -> p h c", h=HEADS),
                        in0=xr_self[:, b, :].rearrange("p (h c) -> p h c", h=HEADS),
                        in1=ex_t[:, 0, :, None].broadcast_to([P, HEADS, CH]), op=OP.mult)
                    nc.vector.tensor_tensor(
                        out=v_t[:, 1:SUBSb, :HC].rearrange("p j (h c) -> p j h c", h=HEADS),
                        in0=xj_t[:, :NGb, :].rearrange("p j (h c) -> p j h c", h=HEADS),
                        in1=ex_t[:, 1:SUBSb, :, None].broadcast_to([P, NGb, HEADS, CH]), op=OP.mult)
                    nc.vector.tensor_copy(v_t[:, :SUBSb, HC:], ex_t[:, :SUBSb, :])

                    acc = psB.tile([P, HC + HEADS], F32, tag="acc")
                    for j in range(SUBSb):
                        nc.tensor.matmul(acc[:], st_t[:, j, :], v_t[:, j, :],
                                         start=(j == 0), stop=(j == SUBSb - 1))
                    nc.vector.tensor_copy(acc_all[:, b, :], acc[:])

                _post(3 * (NBLK // 4), NBLK)

                hT_prev, hT_new = hT_new, hT_prev
                if li < KDBG_LAYERS - 1:
                    _xr_transform(li + 1, hT_prev, 3 * (NBLK // 4), NBLK, finish=True)

    nc.compile()
    return nc


def _make_in_maps(inputs, meta):
    x = np.asarray(inputs["x"], np.float32)
    common = {
        "win": np.asarray(inputs["W_in"], np.float32),
        "binb": _bcast(inputs["b_in"]),
        "wout": np.asarray(inputs["W_out"], np.float32),
        "boutb": _bcast(inputs["b_out"]),
    }
    for i in range(LAYERS):
        common[f"wl{i}"] = np.asarray(inputs[f"Wl{i}"], np.float32)
        common[f"wr{i}"] = np.asarray(inputs[f"Wr{i}"], np.float32)
        common[f"blb{i}"] = _bcast(inputs[f"bl{i}"])
        common[f"brb{i}"] = _bcast(inputs[f"br{i}"])
        common[f"attb{i}"] = _bcast16(np.asarray(inputs[f"att{i}"], np.float32).reshape(-1))
        common[f"bob{i}"] = _bcast(inputs[f"bo{i}"])
        common[f"gb{i}"] = _bcast(inputs[f"g{i}"])
        common[f"beb{i}"] = _bcast(inputs[f"be{i}"])
    in_maps = []
    for c in range(NCORES):
        m = dict(common)
        xT = np.zeros((IN, NPAD), np.float32)
        xT[:, :NPC] = x[c * NPC:(c + 1) * NPC, :].T
        m["xT"] = xT
        m["ixall"] = meta["ixall"][c]
        m["stoh"] = meta["st_oh"][c]
        m["soh"] = meta["s_oh"][c]
        in_maps.append(m)
    return in_maps


def kernel(**inputs):
    edge_index = np.asarray(inputs["edge_index"])
    meta = _preprocess(edge_index)
    nc = _build(meta)
    in_maps = _make_in_maps(inputs, meta)
    res = run_bass_kernel_spmd(nc, in_maps, list(range(NCORES)))
    out = np.concatenate([res.results[c]["out"] for c in range(NCORES)], axis=0)
    return out.astype(np.float32)
